# revision 2
# baseline (speedup 1.0000x reference)
"""Trainium2 Bass kernel for quaternion capsule routing layer (v2).

Math (reference):
  qn = normalize(quats); votes[n,o,i,:] = scale[o,i]*H(qn[o,i], x[n,i]) + (0,trans)
  3 iterations of dynamic routing (softmax over o, weighted vote sum, squash,
  agreement update), then sigmoid-gated output poses.

Strategy (8 cores, data-parallel over n):
  - votes are a LINEAR map of x: host folds rotation+scale+translation into
    W [65, 1088].  Cols 0..1023 = votes in (d,o,i) order, cols 1024..1087 =
    iteration-1 s (c is uniform 1/16).
  - tiles of 128 capsules processed in lockstep GROUPS of G=4 to amortize
    instruction overhead; 2 software-pipeline slots.
  - einsums on DVE as bf16 2x tensor_tensor products + contiguous fold-trees
    (i-sum) / d-plane tree adds (d-sum).  softmax Z on GpSimd.  exp/ln/square
    + PSUM staging on ACT (single activation table set: exp/ln/square/copy).
"""

import numpy as np

import concourse.bass as bass
import concourse.mybir as mybir
from concourse.tile import TileContext
from concourse import bass_utils

F32 = mybir.dt.float32
BF16 = mybir.dt.bfloat16
AX = mybir.AxisListType
OP = mybir.AluOpType
AF = mybir.ActivationFunctionType

N, I, O, ITERS = 65536, 16, 16, 3
EPS = 1e-8
NCORES = 8
P = 128
N_CORE = N // NCORES            # 8192
NT = N_CORE // P                # 64 tiles per core
G = 4                           # tiles per lockstep group
NGRP = NT // G                  # 16 groups
SLOTS = 3                       # software pipeline slots
SKEW = 4                        # phase offset between pipeline slots

# engine/dtype tuning flags
BF16_EINSUM = True              # bf16 votes/products/folds storage
BF16_SM = False                 # bf16 softmax path (e, zi, zf)
BF16_FOLD = False               # bf16 fold scratch fa
Z_ON_POOL = False              # softmax denominator sum on GpSimd (fold tree)
BM_ON_POOL = True               # b-merge on GpSimd
VEXP_ENGINE = 'act'             # v -> (d,o,i) expansion: 'act'|'pool'|'dve'
C_ON_POOL = True                # c = e*zi on GpSimd
VCOPY_ON_POOL = False           # votes PSUM->SBUF copies on GpSimd
VEXP_SPLIT = False              # expand iter0 on Pool, iter1 on ACT


# --------------------------------------------------------------------------
# host-side parameter folding (same as baseline)
# --------------------------------------------------------------------------
def _build_W(quats, scale, trans):
    """W [65, 1088] f32.  Rows (i*4+e) for e in 0..3 plus ones-row 64.
    Cols 0..1023: votes, col j = d*256 + o*16 + i.
    Cols 1024..1087: iter-1 s (=mean of votes over i), col 1024 + d*16 + o."""
    q = quats.astype(np.float64)
    qn = q / np.sqrt((q * q).sum(-1, keepdims=True) + EPS)
    w, x, y, z = qn[..., 0], qn[..., 1], qn[..., 2], qn[..., 3]
    M = np.stack([
        np.stack([w, -x, -y, -z], -1),
        np.stack([x,  w, -z,  y], -1),
        np.stack([y,  z,  w, -x], -1),
        np.stack([z, -y,  x,  w], -1),
    ], -2)                                    # [O, I, d, e]
    A = scale.astype(np.float64)[..., None] * M     # [O, I, d, e]
    t = np.concatenate([np.zeros(trans.shape[:-1] + (1,)),
                        trans.astype(np.float64)], -1)   # [O, I, d]

    W = np.zeros((65, 1088), np.float64)
    Wv = W[:, :1024].reshape(65, 4, O, I)       # [row, d, o, i]
    for i in range(I):
        Wv[i * 4:(i + 1) * 4, :, :, i] = A[:, i, :, :].transpose(2, 1, 0)  # [e,d,o]
    Wv[64, :, :, :] = t.transpose(2, 0, 1)      # [d, o, i] <- t[o,i,d]
    Ws = W[:, 1024:].reshape(65, 4, O)
    for i in range(I):
        Ws[i * 4:(i + 1) * 4, :, :] += A[:, i, :, :].transpose(2, 1, 0) / I
    Ws[64, :, :] += t.transpose(2, 0, 1).sum(-1) / I
    return np.ascontiguousarray(W, dtype=np.float32)


# --------------------------------------------------------------------------
# helpers
# --------------------------------------------------------------------------
def _ap(a, off, dims):
    """AP at element offset `off` past AP `a`'s origin with free dims
    [[step, count], ...] (partition dim copied)."""
    return bass.AP(a.tensor, a.offset + off, [list(a.ap[0])] + [list(d) for d in dims])


def _fixup_bir_for_walrus(nc):
    """Adapt Tile/bass output to this container's walrus build (single
    sync-wait slot per ISA struct; no EVENT_SEMAPHORE_RANGE_CLEAR)."""
    import bass_rust as _br
    cnt = 0
    for blk in nc.m.functions[0].blocks:
        out = []
        changed = False
        for ins in blk.instructions:
            si = ins.sync_info
            if si is not None and len(si.on_wait) > 1:
                waits = list(si.on_wait)
                for w in waits[:-1]:
                    cnt += 1
                    nop = mybir.InstNoOp(
                        name=f"I-wsplit-{cnt}", engine=ins.engine,
                        text_hint="wsplit", bass_nofuse=True,
                        ins=[], outs=[],
                        sync_info=_br.SyncInfo(on_wait=[w], on_update=[]))
                    out.append(nop)
                ins.sync_info = _br.SyncInfo(
                    on_wait=[waits[-1]], on_update=list(si.on_update))
                changed = True
            if (type(ins).__name__ == "InstISA"
                    and getattr(ins, "ant_dict", None)
                    and ins.ant_dict.get("header", {}).get("opcode") == 176):
                lo = ins.ant_dict["range_first"]
                hi = ins.ant_dict["range_last"]
                base_si = ins.sync_info
                for k, sem in enumerate(range(lo, hi + 1)):
                    cnt += 1
                    upd = _br.SyncUpdate(
                        sync_type="semaphore", id=sem,
                        update_mode="sem-wr-imm", update_value=0)
                    ev = mybir.InstEventSemaphore(
                        name=f"I-semclr-{cnt}", engine=ins.engine,
                        ins=[], outs=[],
                        sync_info=_br.SyncInfo(
                            on_wait=list(base_si.on_wait) if (k == 0 and base_si) else [],
                            on_update=[upd]))
                    out.append(ev)
                changed = True
                continue
            out.append(ins)
        if changed:
            blk.instructions = out
    return cnt


# --------------------------------------------------------------------------
# device kernel
# --------------------------------------------------------------------------
def _build_nc():
    nc = bass.Bass(trn_type="TRN2")
    x_d = nc.dram_tensor("x", [N_CORE, 65], F32, kind="ExternalInput")
    W_d = nc.dram_tensor("W", [65, 1088], F32, kind="ExternalInput")
    bb_d = nc.dram_tensor("bb", [P, O], F32, kind="ExternalInput")   # beta bcast
    ab_d = nc.dram_tensor("ab", [P, O], F32, kind="ExternalInput")   # alpha+bias
    id_d = nc.dram_tensor("ident", [P, P], F32, kind="ExternalInput")
    y_d = nc.dram_tensor("y", [N_CORE, 64], F32, kind="ExternalOutput")

    EDT = BF16 if BF16_EINSUM else F32
    SDT = BF16 if BF16_SM else F32
    FDT = BF16 if BF16_FOLD else F32

    with TileContext(nc) as tc, \
         tc.tile_pool(name="const", bufs=1) as cpool, \
         tc.tile_pool(name="stage", bufs=1) as spool, \
         tc.tile_pool(name="lhs", bufs=3) as lpool, \
         tc.tile_pool(name="slot", bufs=1) as slpool, \
         tc.tile_pool(name="px", bufs=2, space="PSUM") as px, \
         tc.tile_pool(name="pv", bufs=2, space="PSUM") as pv:

        W_sb = cpool.tile([65, 1088], F32, tag="W")
        id_sb = cpool.tile([P, P], F32, tag="ident")
        bb_sb = cpool.tile([P, O], F32, tag="bb")
        ab_sb = cpool.tile([P, O], F32, tag="ab")
        eps_sb = cpool.tile([P, 1], F32, tag="eps")
        nc.vector.memset(eps_sb[:, :], EPS)
        nc.sync.dma_start(out=W_sb[:, :], in_=W_d[:, :])
        nc.sync.dma_start(out=id_sb[:, :], in_=id_d[:, :])
        nc.sync.dma_start(out=bb_sb[:, :], in_=bb_d[:, :])
        nc.sync.dma_start(out=ab_sb[:, :], in_=ab_d[:, :])



        # Prologue: PE ops absorbing one DMA-lane wait each.
        pa = px.tile([P, P], F32, tag="xt")
        nc.tensor.transpose(pa[:, :], id_sb[:, :], id_sb[:, :])
        pb = px.tile([P, P], F32, tag="xt")
        nc.tensor.transpose(pb[:, :], W_sb[:, 0:P], id_sb[0:65, :])
        pc = px.tile([P, P], F32, tag="xt")
        nc.tensor.transpose(pc[:, :], W_sb[:, 128:256], id_sb[0:65, :])

        GV = G * 1024
        GE = G * 256
        GS = G * 64
        GO = G * 16

        def emit_group(gg, sfx):
            t0 = gg * G
            xs = slpool.tile([P, G * 65], F32, tag="xs" + sfx)
            nc.sync.dma_start(
                out=xs[:, :].rearrange("p (t f) -> p t f", f=65),
                in_=x_d[t0 * P:(t0 + G) * P, :].rearrange("(t p) f -> p t f", p=P),
            )
            # ---- slot buffers ----
            V = slpool.tile([P, GV], EDT, tag="V" + sfx)        # votes (d,o,i)
            s1sb = slpool.tile([P, GS], F32, tag="s1" + sfx)    # iter0 s
            PR = slpool.tile([P, GV], EDT, tag="PR" + sfx)      # products
            VE = slpool.tile([P, GV], EDT, tag="VE" + sfx)      # expanded v
            fa = slpool.tile([P, G * 512], FDT, tag="fa" + sfx)
            fb = slpool.tile([P, G * 256], EDT, tag="fb" + sfx)
            fc = slpool.tile([P, G * 128], EDT, tag="fc" + sfx)
            t01 = slpool.tile([P, GE], EDT, tag="t01" + sfx)
            t23 = slpool.tile([P, GE], EDT, tag="t23" + sfx)
            b0 = slpool.tile([P, GE], EDT, tag="b0" + sfx)
            b1 = slpool.tile([P, GE], EDT, tag="b1" + sfx)
            e_t = slpool.tile([P, GE], SDT, tag="e" + sfx)
            c_t = slpool.tile([P, GE], EDT, tag="c" + sfx)
            Z_t = slpool.tile([P, GO], F32, tag="Z" + sfx)
            zi_t = slpool.tile([P, GO], SDT, tag="zi" + sfx)
            if Z_ON_POOL:
                zf_t = slpool.tile([P, G * 128], SDT, tag="zf" + sfx)
            else:
                zf_t = None
            s_t = slpool.tile([P, GS], F32, tag="s" + sfx)
            sq_t = slpool.tile([P, GS], F32, tag="sq" + sfx)
            n2_t = slpool.tile([P, GO], F32, tag="n2" + sfx)
            u_t = slpool.tile([P, GO], F32, tag="u" + sfx)
            w_t = slpool.tile([P, GO], F32, tag="w" + sfx)
            ln_t = slpool.tile([P, GO], F32, tag="ln" + sfx)
            r_t = slpool.tile([P, GO], F32, tag="r" + sfx)
            f_t = slpool.tile([P, GO], F32, tag="f" + sfx)
            v_t = slpool.tile([P, GS], F32, tag="v" + sfx)
            out_t = slpool.tile([P, GS], F32, tag="out" + sfx)

            # ---- PE: per-tile transpose + matmuls into PSUM ----
            vp = []
            for g in range(G):
                t = t0 + g
                xt = px.tile([65, P], F32, tag="xt")
                nc.tensor.transpose(xt[:, :], xs[:, g * 65:(g + 1) * 65], id_sb[:, :])
                lhs = lpool.tile([65, P], F32, tag="lhs")
                nc.scalar.copy(lhs[:, :], xt[:, :])
                vps = pv.tile([P, 1088], F32, tag="votes")
                nc.tensor.matmul(vps[:, 0:512], lhs[:, :], W_sb[:, 0:512],
                                 start=True, stop=True)
                nc.tensor.matmul(vps[:, 512:1024], lhs[:, :], W_sb[:, 512:1024],
                                 start=True, stop=True)
                nc.tensor.matmul(vps[:, 1024:1088], lhs[:, :], W_sb[:, 1024:1088],
                                 start=True, stop=True)
                vp.append(vps)
            yield
            # ---- stage votes (cast) + s1 out of PSUM ----
            for g in range(G):
                if VCOPY_ON_POOL:
                    nc.gpsimd.tensor_scalar_add(
                        V[:, g * 1024:(g + 1) * 1024], vp[g][:, 0:1024], 0.0)
                else:
                    nc.scalar.copy(V[:, g * 1024:(g + 1) * 1024], vp[g][:, 0:1024])
                nc.scalar.copy(s1sb[:, g * 64:(g + 1) * 64], vp[g][:, 1024:1088])
                if g == 1:
                    yield
            yield

            def squash(s_flat_ap):
                """sq/n2/u/w/ln/r/f from flat [P, GS] s in (d,o) order."""
                nc.scalar.activation(sq_t[:, :], s_flat_ap, AF.Square)
                nc.vector.tensor_reduce(
                    out=_ap(n2_t[:, :], 0, [[16, G], [1, 16]]),
                    in_=_ap(sq_t[:, :], 0, [[64, G], [1, 16], [16, 4]]),
                    op=OP.add, axis=AX.X)
                nc.vector.tensor_scalar_add(u_t[:, :], n2_t[:, :], 1.0)
                nc.vector.reciprocal(w_t[:, :], u_t[:, :])
                nc.scalar.activation(ln_t[:, :], n2_t[:, :], AF.Ln, bias=eps_sb[:, :])
                nc.scalar.activation(r_t[:, :], ln_t[:, :], AF.Exp, scale=-0.5)
                nc.vector.tensor_tensor(out=f_t[:, :], in0=n2_t[:, :],
                                        in1=w_t[:, :], op=OP.mult)
                nc.vector.tensor_tensor(out=f_t[:, :], in0=f_t[:, :],
                                        in1=r_t[:, :], op=OP.mult)

            def vexpand(s_flat, it=1):
                """v = s*f (d,o); VE[(d,o,i)] = v[(d,o)] broadcast over i."""
                nc.vector.tensor_tensor(
                    out=_ap(v_t[:, :], 0, [[64, G], [16, 4], [1, 16]]),
                    in0=_ap(s_flat, 0, [[64, G], [16, 4], [1, 16]]),
                    in1=_ap(f_t[:, :], 0, [[16, G], [0, 4], [1, 16]]),
                    op=OP.mult)
                H = G // 2
                eng_name = VEXP_ENGINE
                if VEXP_SPLIT:
                    eng_name = 'pool' if it == 0 else 'act'
                for h in range(2):
                    if eng_name == 'act':
                        nc.scalar.copy(
                            _ap(VE[:, :], h * H * 1024, [[1024, H], [16, 64], [1, 16]]),
                            _ap(v_t[:, :], h * H * 64, [[64, H], [1, 64], [0, 16]]))
                    else:
                        eng = nc.gpsimd if eng_name == 'pool' else nc.vector
                        eng.tensor_scalar_add(
                            _ap(VE[:, :], h * H * 1024, [[1024, H], [16, 64], [1, 16]]),
                            _ap(v_t[:, :], h * H * 64, [[64, H], [1, 64], [0, 16]]), 0.0)

            def t2_einsum(dst):
                """dst[(o,i)] = sum_d votes*VE  (products + d-plane tree)."""
                H2 = G * 512
                nc.vector.tensor_tensor(out=PR[:, 0:H2], in0=V[:, 0:H2],
                                        in1=VE[:, 0:H2], op=OP.mult)
                nc.vector.tensor_tensor(out=PR[:, H2:], in0=V[:, H2:],
                                        in1=VE[:, H2:], op=OP.mult)
                nc.vector.tensor_tensor(
                    out=_ap(t01[:, :], 0, [[256, G], [1, 256]]),
                    in0=_ap(PR[:, :], 0, [[1024, G], [1, 256]]),
                    in1=_ap(PR[:, :], 256, [[1024, G], [1, 256]]),
                    op=OP.add)
                nc.vector.tensor_tensor(
                    out=_ap(t23[:, :], 0, [[256, G], [1, 256]]),
                    in0=_ap(PR[:, :], 512, [[1024, G], [1, 256]]),
                    in1=_ap(PR[:, :], 768, [[1024, G], [1, 256]]),
                    op=OP.add)
                nc.vector.tensor_tensor(out=dst[:, :], in0=t01[:, :],
                                        in1=t23[:, :], op=OP.add)

            def t1_einsum():
                """s[(d,o)] = sum_i votes*c  (products + fold tree over i)."""
                nc.vector.tensor_tensor(
                    out=_ap(PR[:, :], 0, [[1024, G], [256, 4], [1, 256]]),
                    in0=_ap(V[:, :], 0, [[1024, G], [256, 4], [1, 256]]),
                    in1=_ap(c_t[:, :], 0, [[256, G], [0, 4], [1, 256]]),
                    op=OP.mult)
                nc.vector.tensor_tensor(
                    out=_ap(fa[:, :], 0, [[512, G], [8, 64], [1, 8]]),
                    in0=_ap(PR[:, :], 0, [[1024, G], [16, 64], [1, 8]]),
                    in1=_ap(PR[:, :], 8, [[1024, G], [16, 64], [1, 8]]),
                    op=OP.add)
                nc.vector.tensor_tensor(
                    out=_ap(fb[:, :], 0, [[256, G], [4, 64], [1, 4]]),
                    in0=_ap(fa[:, :], 0, [[512, G], [8, 64], [1, 4]]),
                    in1=_ap(fa[:, :], 4, [[512, G], [8, 64], [1, 4]]),
                    op=OP.add)
                nc.vector.tensor_tensor(
                    out=_ap(fc[:, :], 0, [[128, G], [2, 64], [1, 2]]),
                    in0=_ap(fb[:, :], 0, [[256, G], [4, 64], [1, 2]]),
                    in1=_ap(fb[:, :], 2, [[256, G], [4, 64], [1, 2]]),
                    op=OP.add)
                nc.vector.tensor_tensor(
                    out=_ap(s_t[:, :], 0, [[64, G], [1, 64]]),
                    in0=_ap(fc[:, :], 0, [[128, G], [2, 64]]),
                    in1=_ap(fc[:, :], 1, [[128, G], [2, 64]]),
                    op=OP.add)

            def softmax(b_src):
                nc.scalar.activation(e_t[:, :], b_src, AF.Exp)
                if Z_ON_POOL:
                    # fold tree over o (contiguous o-halves) on GpSimd
                    nc.gpsimd.tensor_tensor(
                        out=_ap(zf_t[:, :], 0, [[128, G], [1, 128]]),
                        in0=_ap(e_t[:, :], 0, [[256, G], [1, 128]]),
                        in1=_ap(e_t[:, :], 128, [[256, G], [1, 128]]),
                        op=OP.add)
                    nc.gpsimd.tensor_tensor(
                        out=_ap(zf_t[:, :], 0, [[128, G], [1, 64]]),
                        in0=_ap(zf_t[:, :], 0, [[128, G], [1, 64]]),
                        in1=_ap(zf_t[:, :], 64, [[128, G], [1, 64]]),
                        op=OP.add)
                    nc.gpsimd.tensor_tensor(
                        out=_ap(zf_t[:, :], 0, [[128, G], [1, 32]]),
                        in0=_ap(zf_t[:, :], 0, [[128, G], [1, 32]]),
                        in1=_ap(zf_t[:, :], 32, [[128, G], [1, 32]]),
                        op=OP.add)
                    nc.gpsimd.tensor_tensor(
                        out=_ap(Z_t[:, :], 0, [[16, G], [1, 16]]),
                        in0=_ap(zf_t[:, :], 0, [[128, G], [1, 16]]),
                        in1=_ap(zf_t[:, :], 16, [[128, G], [1, 16]]),
                        op=OP.add)
                else:
                    nc.vector.tensor_reduce(
                        out=_ap(Z_t[:, :], 0, [[16, G], [1, 16]]),
                        in_=_ap(e_t[:, :], 0, [[256, G], [1, 16], [16, 16]]),
                        op=OP.add, axis=AX.X)
                with nc.allow_low_precision(reason="bf16 softmax weights"):
                    nc.vector.reciprocal(zi_t[:, :], Z_t[:, :])
                ceng = nc.gpsimd if C_ON_POOL else nc.vector
                ceng.tensor_tensor(
                    out=_ap(c_t[:, :], 0, [[256, G], [16, 16], [1, 16]]),
                    in0=_ap(e_t[:, :], 0, [[256, G], [16, 16], [1, 16]]),
                    in1=_ap(zi_t[:, :], 0, [[16, G], [0, 16], [1, 16]]),
                    op=OP.mult)

            # ================= iter 0 =================
            squash(s1sb[:, :])
            yield
            vexpand(s1sb[:, :], it=0)
            yield
            t2_einsum(b0)          # b after iter0 = delta0
            yield
            # ================= iter 1 =================
            softmax(b0[:, :])
            yield
            t1_einsum()
            yield
            squash(s_t[:, :])
            yield
            vexpand(s_t[:, :])
            yield
            t2_einsum(b1)          # delta1
            bmeng = nc.gpsimd if BM_ON_POOL else nc.vector
            bmeng.tensor_tensor(out=b1[:, :], in0=b0[:, :], in1=b1[:, :], op=OP.add)
            yield
            # ================= iter 2 =================
            softmax(b1[:, :])
            yield
            t1_einsum()
            yield
            squash(s_t[:, :])
            yield
            # ================= gate =================
            #   norm ~= n2 * rsqrt(n2+eps) = n2 * r;  z = beta*norm + (alpha+bias)
            #   a = 1/(1+exp(-z));  out[(o,d)] = s[(d,o)] * (f*a)[o]
            nc.vector.tensor_tensor(out=u_t[:, :], in0=n2_t[:, :], in1=r_t[:, :],
                                    op=OP.mult)
            nc.vector.tensor_tensor(
                out=w_t[:, :], in0=u_t[:, :],
                in1=_ap(bb_sb[:, :], 0, [[0, G], [1, 16]]), op=OP.mult)
            nc.vector.tensor_tensor(
                out=u_t[:, :], in0=w_t[:, :],
                in1=_ap(ab_sb[:, :], 0, [[0, G], [1, 16]]), op=OP.add)
            nc.vector.tensor_scalar(out=w_t[:, :], in0=u_t[:, :],
                                    scalar1=-87.0, scalar2=87.0,
                                    op0=OP.max, op1=OP.min)
            nc.scalar.activation(ln_t[:, :], w_t[:, :], AF.Exp, scale=-1.0)
            nc.vector.tensor_scalar_add(u_t[:, :], ln_t[:, :], 1.0)
            nc.vector.reciprocal(w_t[:, :], u_t[:, :])
            nc.vector.tensor_tensor(out=f_t[:, :], in0=f_t[:, :], in1=w_t[:, :],
                                    op=OP.mult)
            nc.vector.tensor_tensor(
                out=_ap(out_t[:, :], 0, [[64, G], [1, 4], [4, 16]]),
                in0=_ap(s_t[:, :], 0, [[64, G], [16, 4], [1, 16]]),
                in1=_ap(f_t[:, :], 0, [[16, G], [0, 4], [1, 16]]),
                op=OP.mult)
            rows = y_d[t0 * P:(t0 + G) * P, :]
            nc.sync.dma_start(
                out=rows.rearrange("(t p) f -> p t f", p=P),
                in_=out_t[:, :].rearrange("p (t f) -> p t f", f=64),
            )
            yield

        sfxs = ["A", "B", "C", "D"]
        free_slots = list(range(SLOTS))
        active = []          # [gen, steps]
        next_g = 0

        def spawn():
            nonlocal next_g
            if (next_g < NGRP and free_slots
                    and (not active or active[-1][1] >= SKEW)):
                slot = free_slots.pop(0)
                active.append([emit_group(next_g, sfxs[slot]), 0, slot])
                next_g += 1

        spawn()
        while active:
            spawn()
            for ent in list(active):
                try:
                    next(ent[0])
                    ent[1] += 1
                except StopIteration:
                    active.remove(ent)
                    free_slots.append(ent[2])
    _fixup_bir_for_walrus(nc)
    return nc


_NC_CACHE = None


def _get_nc():
    global _NC_CACHE
    if _NC_CACHE is None:
        _NC_CACHE = _build_nc()
    return _NC_CACHE


class _Runner:
    """Cached shard_map-jitted executor over the 8 cores."""

    def __init__(self):
        import jax
        from jax.experimental.shard_map import shard_map
        from jax.sharding import Mesh, PartitionSpec, NamedSharding
        from concourse.bass2jax import (
            _bass_exec_p, install_neuronx_cc_hook, partition_id_tensor)

        install_neuronx_cc_hook()
        nc = _get_nc()
        in_names, out_names, out_avals = [], [], []
        import concourse.mybir as _mb
        pid_name = nc.partition_id_tensor.name if nc.partition_id_tensor else None
        for alloc in nc.m.functions[0].allocations:
            if not isinstance(alloc, _mb.MemoryLocationSet):
                continue
            name = alloc.memorylocations[0].name
            if alloc.kind == "ExternalInput":
                if name != pid_name:
                    in_names.append(name)
            elif alloc.kind == "ExternalOutput":
                out_names.append(name)
                out_avals.append(jax.core.ShapedArray(
                    tuple(alloc.tensor_shape), _mb.dt.np(alloc.dtype)))
        self.in_names, self.out_names, self.out_avals = in_names, out_names, out_avals
        n_params, n_outs = len(in_names), len(out_names)
        all_names = list(in_names) + list(out_names)
        if pid_name is not None:
            all_names.append(pid_name)

        def _body(*args):
            operands = list(args)
            if pid_name is not None:
                operands.append(partition_id_tensor())
            outs = _bass_exec_p.bind(
                *operands,
                out_avals=tuple(out_avals),
                in_names=tuple(all_names),
                out_names=tuple(out_names),
                lowering_input_output_aliases=(),
                sim_require_finite=True,
                sim_require_nnan=True,
                nc=nc,
            )
            return tuple(outs)

        devices = jax.devices()[:NCORES]
        self.mesh = Mesh(np.asarray(devices), ("core",))
        self.pspec = PartitionSpec("core")
        self.sharding = NamedSharding(self.mesh, self.pspec)
        in_specs = (self.pspec,) * (n_params + n_outs)
        out_specs = (self.pspec,) * n_outs
        self.fn = jax.jit(
            shard_map(_body, mesh=self.mesh, in_specs=in_specs,
                      out_specs=out_specs, check_rep=False),
            donate_argnums=tuple(range(n_params, n_params + n_outs)),
            keep_unused=True,
        )
        self._jax = jax

    def zeros(self):
        import jax
        import jax.numpy as jnp
        if not hasattr(self, "_zfn"):
            avals = self.out_avals
            self._zfn = jax.jit(
                lambda: tuple(
                    jnp.zeros((NCORES * a.shape[0], *a.shape[1:]), a.dtype)
                    for a in avals),
                out_shardings=tuple(self.sharding for _ in avals))
        return list(self._zfn())

    def run(self, concat_inputs):
        outs = self.fn(*concat_inputs, *self.zeros())
        return [np.asarray(o) for o in outs]


_RUNNER = None


def _get_runner():
    global _RUNNER
    if _RUNNER is None:
        _RUNNER = _Runner()
    return _RUNNER


def _prep_inputs(x, quats, scale, trans, bias=None, beta=None, alpha=None):
    x = np.asarray(x, np.float32)
    W = _build_W(np.asarray(quats), np.asarray(scale), np.asarray(trans))
    bb = np.tile(np.asarray(beta, np.float32)[None, :], (P, 1))
    ab = np.tile((np.asarray(alpha, np.float32)
                  + np.asarray(bias, np.float32))[None, :], (P, 1))
    ident = np.eye(P, dtype=np.float32)
    x_aug = np.empty((N, 65), np.float32)
    x_aug[:, :64] = x.reshape(N, 64)
    x_aug[:, 64] = 1.0
    per_core = {
        "x": x_aug,
        "W": np.concatenate([W] * NCORES, axis=0),
        "bb": np.concatenate([bb] * NCORES, axis=0),
        "ab": np.concatenate([ab] * NCORES, axis=0),
        "ident": np.concatenate([ident] * NCORES, axis=0),
    }
    r = _get_runner()
    return [per_core[name] for name in r.in_names]


def kernel(x, quats, scale, trans, bias, beta, alpha):
    r = _get_runner()
    concat_in = _prep_inputs(x, quats, scale, trans,
                             bias=bias, beta=beta, alpha=alpha)
    outs = r.run(concat_in)
    y = outs[r.out_names.index("y")]
    return np.ascontiguousarray(y.reshape(N, O, 4).astype(np.float32))
